# revision 41
# baseline (speedup 1.0000x reference)
"""Trainium2 Bass kernel for fused QKV projection + interleaved RoPE.

Problem: X[4, 4096, 2048] @ {Wq, Wk, Wv}[2048, 2048] -> reshape to heads
[B, S, 16, 128], apply interleaved RoPE to Q and K, return (Xq, Xk, Xv).

Sharding: data-parallel over tokens. The 4*4096 = 16384 token rows are
split into 8 contiguous shards of 2048 rows (core c gets batch c//2,
sequence half c%2). Every core holds the full Wq/Wk/Wv and computes all
2048 output features for its rows; RoPE is per-token elementwise so no
communication is needed.

Device kernel (identical SPMD program on all 8 cores):
  - Mixed-precision contraction: the first NF8 (=4) of 16 k-chunks are
    computed in fp8e4 with MatmulPerfMode.DoubleRow (contracts 256 rows
    per instruction at the same per-instruction cost as one bf16
    k-chunk), the remaining 12 chunks in bf16. Both accumulate into the
    same fp32 PSUM group, cutting PE instruction count per output tile
    from 32 to 28 (-12.5%). Host-measured rel err ~1.9e-2 < 2e-2 gate.
    fp8 operand scales: X*2**-3, W*2**3 (product unscaled) keep both
    operand distributions clear of the e4m3 denormal floor.
  - A burst of dummy N=64 warmup matmuls runs while the first DMAs
    land, so the PE HAM clock-gate (default K=4/8, 1.2 GHz) is already
    released to full rate when the real matmul stream starts.
  - Row-chunks are processed in interleaved batches (4 per batch, 6 for
    the phase-0 lead batch): one fp8<->bf16 PE mode transition per
    batch, and the lead interleave keeps phase-0's W-tile demand below
    the early HBM delivery rate. The last 4 k-chunks of each row-chunk
    are peeled per-rc so finish work overlaps the tail matmuls.
  - Phase order Q, K, V (12 phases of 512 output columns): V needs no
    RoPE, so the kernel tail after the last matmul is a copy + store
    instead of the 3-op DVE RoPE chain. DMAs are issued in consumption
    order on two HWDGE rings (SP: activations/freqs, ACT: weights +
    stores) with few grouped W dma_starts per phase (sequencer issue
    costs ~667ns each).
  - matmul out = lhsT.T @ rhs with lhsT = X^T tile (stationary) and
    rhs = W tile (moving), accumulating psum[128r, 512m] fp32.
  - finish: single PSUM read via ACT copy to SBUF, then RoPE in 3 DVE
    ops on SBUF: the interleaved pair swap is a reversed-stride access
    pattern, the rotation sign is pre-baked into the sin table on the
    host, and cos/sin broadcast across heads via zero-stride APs.
"""

import numpy as np
import ml_dtypes

import concourse.mybir as mybir
import concourse.tile as tile
from concourse import bacc
from concourse.bass import ds, ts
from concourse.bass_utils import run_bass_kernel_spmd

B, S, DIM, H = 4, 4096, 2048, 16
HD = DIM // H           # 128
N_CORES = 8
R = B * S // N_CORES    # 2048 token rows per core
P = 128

NF8 = 4                 # k-chunks computed in fp8 DoubleRow (must be even)
NPAIR = NF8 // 2
K_BF = DIM - NF8 * P    # bf16 k-rows
KO_BF = K_BF // P       # bf16 k-chunks
SX = 0.125              # fp8 X scale; W gets 1/SX so products are unscaled

BF16 = mybir.dt.bfloat16
F32 = mybir.dt.float32
F8 = mybir.dt.float8e4


def build_nc(M=DIM, rows=R, hd=HD, mm_free=512, m_half=512, n_warm=96,
             n_lead=6):
    m_half = min(m_half, M)
    assert rows % P == 0 and M % m_half == 0
    assert m_half % mm_free == 0 and m_half % hd == 0
    RC = rows // P        # token row chunks
    HALVES = M // m_half  # weight column phases per tensor
    MJ = m_half // mm_free
    J = hd // 2           # rotation pairs per head

    nc = bacc.Bacc(None, target_bir_lowering=False)

    RC_ = rows // P
    # rc-major layouts so each row-chunk's slice is one contiguous DMA
    xt8 = nc.dram_tensor("xt8", [P, RC_, NPAIR, 2, P], F8,
                         kind="ExternalInput")
    xtb = nc.dram_tensor("xtb", [P, RC_, KO_BF, P], BF16,
                         kind="ExternalInput")
    w8s = {n: nc.dram_tensor(n + "8", [P, NPAIR, 2, M], F8, kind="ExternalInput")
           for n in ("wq", "wk", "wv")}
    wbs = {n: nc.dram_tensor(n, [K_BF, M], BF16, kind="ExternalInput")
           for n in ("wq", "wk", "wv")}
    cosf = nc.dram_tensor("cosf", [rows, hd], F32, kind="ExternalInput")
    ssin = nc.dram_tensor("ssin", [rows, hd], F32, kind="ExternalInput")
    q_out = nc.dram_tensor("q", [rows, M], F32, kind="ExternalOutput")
    k_out = nc.dram_tensor("k", [rows, M], F32, kind="ExternalOutput")
    v_out = nc.dram_tensor("v", [rows, M], F32, kind="ExternalOutput")

    cos_r = cosf[:].rearrange("(rc p) d -> p rc d", p=P)
    sin_r = ssin[:].rearrange("(rc p) d -> p rc d", p=P)

    with tile.TileContext(nc) as tc:
        with (
            tc.tile_pool(name="warm", bufs=1) as warmp,
            tc.tile_pool(name="wpool", bufs=2) as wpool,
            tc.tile_pool(name="w8pool", bufs=2) as w8pool,
            tc.tile_pool(name="x8pool", bufs=1) as x8pool,
            tc.tile_pool(name="xpool", bufs=RC) as xpool,
            tc.tile_pool(name="cpool", bufs=1) as cpool,
            tc.tile_pool(name="opool", bufs=6) as opool,
            tc.tile_pool(name="tpool", bufs=4) as tpool,
            tc.tile_pool(name="psum", bufs=16 * 1024 // (4 * m_half),
                         space="PSUM") as pspool,
        ):
            def emit_warmup():
                # Dummy matmuls on a zeroed tile keep the PE busy while the
                # prologue DMAs land: the HAM activity window fires during
                # the prologue instead of eating into the real stream.
                # The tile is read uninitialized on purpose: garbage (even
                # NaN) products land in a PSUM slot nothing reads, and
                # skipping the memset lets the PE start at sequencer-boot.
                wz = warmp.tile([P, P], BF16, tag="wz")
                nc.vector.memset(wz[:, :1], 0)
                pw = pspool.tile([P, 64], F32, tag="ps", name="pw")
                for g in range(n_warm // 8):
                    for i in range(8):
                        nc.tensor.matmul(
                            pw[:], wz[:], wz[:, :64],
                            start=(i == 0), stop=(i == 7),
                        )

            def load_w_tiles(wb_r, w8_d, half, splits=(6, 12)):
                # Few grouped DMAs per phase: each dma_start costs ~667ns of
                # ACT sequencer time, so 13 per-ko loads serialize ~9us of
                # issue latency at cold start. Groups keep issue cost low;
                # subtile deps let matmuls start as each group lands.
                w8_sb = w8pool.tile([P, NPAIR, 2, m_half], F8, tag="w8")
                nc.scalar.dma_start(
                    w8_sb[:], w8_d[:, :, :, ts(half, m_half)])
                w_sb = wpool.tile([P, KO_BF, m_half], BF16, tag="w")
                lo = 0
                for hi in splits:
                    nc.scalar.dma_start(
                        w_sb[:, lo:hi], wb_r[:, lo:hi, ts(half, m_half)])
                    lo = hi
                return w8_sb, w_sb

            def emit_batch(rcs, w8_sb, w_tiles, o_r, half, rope, xt8_t,
                           xtb_tiles, cos_sb, sin_sb, peel=4):
                # A batch interleaves several row-chunks' accumulations:
                # all fp8 DoubleRow matmuls first (one bf16<->fp8 PE mode
                # transition per batch instead of per group), then the bf16
                # k-chunks ko-major across the batch; the last `peel` ko
                # are emitted per-rc so early rcs' finish work overlaps the
                # tail matmuls. For the phase-0 lead batch the interleave
                # also slows per-W-tile demand below HBM delivery rate.
                pss = [
                    pspool.tile([P, m_half], F32, tag="ps", name=f"ps_{rc}")
                    for rc in rcs
                ]
                for j in range(NPAIR):
                    for b, rc in enumerate(rcs):
                        for mj in range(MJ):
                            nc.tensor.matmul(
                                pss[b][:, ts(mj, mm_free)],
                                xt8_t[:, rc, j],
                                w8_sb[:, j, :, ts(mj, mm_free)],
                                start=(j == 0), stop=False,
                                perf_mode=mybir.MatmulPerfMode.DoubleRow,
                            )
                for ko in range(KO_BF - peel):
                    for b, rc in enumerate(rcs):
                        for mj in range(MJ):
                            nc.tensor.matmul(
                                pss[b][:, ts(mj, mm_free)],
                                xtb_tiles[rc][:, ko],
                                w_tiles[:, ko, ts(mj, mm_free)],
                                start=False, stop=False,
                            )
                for b, rc in enumerate(rcs):
                    for ko in range(KO_BF - peel, KO_BF):
                        for mj in range(MJ):
                            nc.tensor.matmul(
                                pss[b][:, ts(mj, mm_free)],
                                xtb_tiles[rc][:, ko],
                                w_tiles[:, ko, ts(mj, mm_free)],
                                start=False, stop=(ko == KO_BF - 1),
                            )
                    finish_rc(pss[b], o_r, half, rc, rope, cos_sb, sin_sb)

            def emit_phase(w8_sb, w_tiles, o_r, half, rope, xt8_t, xtb_tiles,
                           cos_sb, sin_sb, lead=0):
                rc0 = 0
                sizes = []
                if lead > 1:
                    sizes.append(lead)
                    rc0 = lead
                while rc0 < RC:
                    b = min(4, RC - rc0)
                    sizes.append(b)
                    rc0 += b
                rc0 = 0
                for b in sizes:
                    emit_batch(list(range(rc0, rc0 + b)), w8_sb, w_tiles,
                               o_r, half, rope, xt8_t, xtb_tiles, cos_sb,
                               sin_sb)
                    rc0 += b

            def finish_rc(psum, o_r, half, rc, rope, cos_sb, sin_sb):
                nh = m_half // hd
                ps = psum[:, ds(0, m_half)]
                o_sb = opool.tile([P, m_half], F32, tag="o")
                # Single PSUM read on the ACT engine (idle during Q/K
                # phases): halves PSUM read-port contention with the PE's
                # writes and frees the PSUM slot as soon as the copy is
                # done; RoPE then runs on SBUF only.
                nc.scalar.copy(o_sb[:], ps)
                if rope:
                    # o = x*cos + swap_pairs(x)*ssin; ssin sign-baked,
                    # the swap is a reversed-stride AP on the pair dim.
                    cos_b = cos_sb[:, rc, None, :].to_broadcast([P, nh, hd])
                    sin_b = sin_sb[:, rc].rearrange(
                        "p (j two) -> p j two", two=2
                    )[:, None, :, :].to_broadcast([P, nh, J, 2])

                    t_sb = tpool.tile([P, m_half], F32, tag="t")
                    t_pr = t_sb[:].rearrange(
                        "p (h j two) -> p h j two", h=nh, two=2
                    )
                    o_pr = o_sb[:].rearrange(
                        "p (h j two) -> p h j two", h=nh, two=2
                    )
                    o_hd = o_sb[:].rearrange("p (h d) -> p h d", d=hd)

                    nc.vector.tensor_tensor(
                        t_pr[:], o_pr[:, :, :, ::-1], sin_b,
                        mybir.AluOpType.mult,
                    )
                    nc.vector.tensor_tensor(
                        o_hd, o_hd, cos_b, mybir.AluOpType.mult,
                    )
                    nc.vector.tensor_tensor(
                        o_sb[:], o_sb[:], t_sb[:], mybir.AluOpType.add,
                    )

                # stores share the ACT HWDGE ring with the W prefetches
                # (now only 3 grouped dma_starts per phase); activations +
                # freqs own the SP ring
                nc.scalar.dma_start(
                    o_r[:, rc, ts(half, m_half)], o_sb[:])

            def body():
                emit_warmup()

                # Cold-start ordering: the first matmuls need only xt8 and
                # the phase-0 fp8 weights, so issue those before everything
                # else (x side on the SP HWDGE ring, W side on ACT's).
                xt8_t = x8pool.tile([P, RC, NPAIR, 2, P], F8, tag="x8")
                nc.sync.dma_start(xt8_t[:, :2], xt8[:, :2])

                phases = []
                for n, o_dram, rope in (
                    ("wq", q_out, True),
                    ("wk", k_out, True),
                    ("wv", v_out, False),  # V last: no RoPE -> short tail
                ):
                    wb_r = wbs[n][:].rearrange("(ko p) m -> p ko m", p=P)
                    o_r = o_dram[:].rearrange("(rc p) m -> p rc m", p=P)
                    for half in range(HALVES):
                        phases.append((wb_r, w8s[n], o_r, half, rope))

                w8_first, w_first = load_w_tiles(
                    phases[0][0], phases[0][1], phases[0][3],
                    splits=(1, 2, 4, 8, 12))

                # SP-ring order = consumption order: the lead chunks' bf16
                # activations, the cos/sin rows the first finishes need,
                # then per-rc (xt8, xtb) pairs just-in-time, with the
                # cos/sin remainder slotted in after the first pair.
                xtb_tiles = []
                for rc in range(2):
                    x_sb = xpool.tile([P, KO_BF, P], BF16, tag="x")
                    nc.sync.dma_start(x_sb[:], xtb[:, rc])
                    xtb_tiles.append(x_sb)
                nc.sync.dma_start(xt8_t[:, 2:n_lead], xt8[:, 2:n_lead])
                for rc in range(2, n_lead):
                    x_sb = xpool.tile([P, KO_BF, P], BF16, tag="x")
                    nc.sync.dma_start(x_sb[:], xtb[:, rc])
                    xtb_tiles.append(x_sb)
                c_split = min(n_lead + 2, RC)
                cos_sb = cpool.tile([P, RC, hd], F32, tag="cos")
                sin_sb = cpool.tile([P, RC, hd], F32, tag="sin")
                nc.sync.dma_start(cos_sb[:, :c_split], cos_r[:, :c_split])
                nc.sync.dma_start(sin_sb[:, :c_split], sin_r[:, :c_split])
                for rc in range(n_lead, RC):
                    nc.sync.dma_start(xt8_t[:, rc], xt8[:, rc])
                    x_sb = xpool.tile([P, KO_BF, P], BF16, tag="x")
                    nc.sync.dma_start(x_sb[:], xtb[:, rc])
                    xtb_tiles.append(x_sb)
                    if rc == n_lead:
                        nc.sync.dma_start(cos_sb[:, c_split:],
                                          cos_r[:, c_split:])
                        nc.sync.dma_start(sin_sb[:, c_split:],
                                          sin_r[:, c_split:])

                for i, (wb_r, w8_d, o_r, half, rope) in enumerate(phases):
                    if i == 0:
                        w8_sb, w_tiles = w8_first, w_first
                    else:
                        w8_sb, w_tiles = load_w_tiles(wb_r, w8_d, half)
                    emit_phase(w8_sb, w_tiles, o_r, half, rope, xt8_t,
                               xtb_tiles, cos_sb, sin_sb,
                               lead=(n_lead if i == 0 else 0))

            body()

    nc.compile()
    return nc


_NC_CACHE = {}


def _get_nc():
    if "nc" not in _NC_CACHE:
        _NC_CACHE["nc"] = build_nc()
    return _NC_CACHE["nc"]


def prepare_in_maps(X, freqs_cos, freqs_sin, Wq, Wk, Wv):
    X = np.asarray(X, dtype=np.float32)
    freqs_cos = np.asarray(freqs_cos, dtype=np.float32)
    freqs_sin = np.asarray(freqs_sin, dtype=np.float32)

    Xf = X.reshape(B * S, DIM)
    KF8 = NF8 * P
    # fp8 slice of X (k < KF8), scaled by SX; pair-of-chunks layout
    # [p, pair, two, row] for DoubleRow's [128, 2, free] stationary AP.
    X8 = (Xf[:, :KF8] * SX).astype(ml_dtypes.float8_e4m3)
    Xb = Xf[:, KF8:].astype(ml_dtypes.bfloat16)

    ws8 = {}
    wsb = {}
    for name, W in (("wq", Wq), ("wk", Wk), ("wv", Wv)):
        W = np.asarray(W, dtype=np.float32)
        W8 = (W[:KF8] * (1.0 / SX)).astype(ml_dtypes.float8_e4m3)
        # [p, pair, two, m]
        ws8[name] = np.ascontiguousarray(
            W8.reshape(NPAIR, 2, P, DIM).transpose(2, 0, 1, 3))
        wsb[name] = W[KF8:].astype(ml_dtypes.bfloat16)

    # Rotation sign baked into sin: out[2i] = x[2i]c - x[2i+1]s,
    # out[2i+1] = x[2i+1]c + x[2i]s.
    ssin_full = freqs_sin.copy()
    ssin_full[:, 0::2] *= -1.0

    RC = R // P
    KO = (DIM - KF8) // P
    in_maps = []
    for c in range(N_CORES):
        rows = slice(c * R, (c + 1) * R)
        s0 = (c % 2) * R  # sequence offset of this shard (R == S // 2)
        # rc-major: xt8[p, rc, pair, two, r], xtb[p, rc, ko, r]
        x8c = X8[rows].reshape(RC, P, NPAIR, 2, P).transpose(4, 0, 2, 3, 1)
        xbc = Xb[rows].reshape(RC, P, KO, P).transpose(3, 0, 2, 1)
        im = {
            "xt8": np.ascontiguousarray(x8c),
            "xtb": np.ascontiguousarray(xbc),
            "cosf": np.ascontiguousarray(freqs_cos[s0:s0 + R]),
            "ssin": np.ascontiguousarray(ssin_full[s0:s0 + R]),
        }
        for name in ("wq", "wk", "wv"):
            im[name + "8"] = ws8[name]
            im[name] = wsb[name]
        in_maps.append(im)
    return in_maps


def assemble_outputs(results):
    Xq = np.empty((B * S, H, HD), dtype=np.float32)
    Xk = np.empty((B * S, H, HD), dtype=np.float32)
    Xv = np.empty((B * S, H, HD), dtype=np.float32)
    for c in range(N_CORES):
        rows = slice(c * R, (c + 1) * R)
        Xq[rows] = results[c]["q"].reshape(R, H, HD)
        Xk[rows] = results[c]["k"].reshape(R, H, HD)
        Xv[rows] = results[c]["v"].reshape(R, H, HD)

    return (
        Xq.reshape(B, S, H, HD),
        Xk.reshape(B, S, H, HD),
        Xv.reshape(B, S, H, HD),
    )


def kernel(X, freqs_cos, freqs_sin, attention_mask, Wq, Wk, Wv):
    in_maps = prepare_in_maps(X, freqs_cos, freqs_sin, Wq, Wk, Wv)
    nc = _get_nc()
    res = run_bass_kernel_spmd(nc, in_maps, list(range(N_CORES)))
    return assemble_outputs(res.results)
